# revision 38
# baseline (speedup 1.0000x reference)
"""Trainium2 Bass kernel for nn_ConvPlus1d (dense_cnn).

Algorithm (mathematically identical to the reference, derived analytically):

  The reference synthesizes per-sample conv weights:
      kern[b]   = mean_L(depthwise_conv(x))        -> [B, C_IN, K]
      w_in[b]   = W_in @ kern[b]                   -> [B, C_IN, K]
      w_out[b]  = <W_out, kern[b]>                 -> [B, C_OUT]
      bias[b]   = <W_bias, kern[b]>                -> [B, C_OUT]
      weight[b, o, c, k] = w_in[b, c, k] * w_out[b, o]     (rank-1!)
      y[b] = conv1d(x[b], weight[b], pad=1) + bias[b]

  Exact simplifications:
  1) mean over L of a pad-1 depthwise conv only needs per-channel sums and
     the first/last elements:  sum_l xpad[c, l+t] = {S-E, S, S-F}[t]
     so kern / w_in / w_out / bias are LINEAR in (S, E, F), with
     coefficient matrices precomputed on the host from maker params.
  2) The per-sample conv weight is rank-1 across (o) x (c,k).

  Device program per sample (data-parallel over batch, 4 samples/core):
      x (bf16) lands in SBUF partitions 0-63; a shifted copy (one column
      left) is DMA'd into partitions 64-127.  The 3-tap conv then needs
      only TWO matmuls per 512-col tile: a 128-contract matmul computes
      taps 0+1 together (stationary [W0; W1]), a 64-contract matmul adds
      tap 2, k-outer over 4-tile groups so consecutive matmuls hit
      different PSUM banks.  Stats -> params synthesis runs in fp32r,
      interleaved into the previous sample's conv stream.  PSUM -> SBUF
      eviction adds the bias and narrows to bf16 (ACT only; DVE owns the
      stats reduces); the output is stored bf16 and widened to f32 on
      the host (~1e-3 extra relative error, half the store traffic);
      stores stream out in 2048-col chunks triggered from GpSimd.

Sharding: batch 32 -> 8 cores x 4 samples, maker params replicated.
"""

import sys

import numpy as np

sys.path.insert(0, "/opt/trn_rl_repo")

import concourse.bacc as bacc  # noqa: E402
import concourse.tile as tile  # noqa: E402
from concourse import mybir  # noqa: E402
from concourse.bass_utils import run_bass_kernel_spmd  # noqa: E402

import ml_dtypes  # noqa: E402

B, C_IN, C_OUT, K, L = 32, 64, 128, 3, 8192
N_CORES = 8
BS = B // N_CORES          # samples per core
NT = 512                   # matmul moving-dim tile (one PSUM bank of fp32)
NTILES = L // NT
NCH = 8                    # partial-reduce windows
CHW = (L + 2) // NCH       # 2048, last chunk takes the +2 remainder

F32 = mybir.dt.float32
F32R = mybir.dt.float32r
BF16 = mybir.dt.bfloat16


def _host_precompute(W_kernel, W_in, W_out, W_bias):
    """Fold the maker parameters into linear maps on the stats (S, E, F)."""
    Wk = W_kernel.reshape(C_IN, K, K).astype(np.float64)     # [c, j, t]
    P = (Wk[:, :, 0] + Wk[:, :, 1] + Wk[:, :, 2]) / L        # coeff on S
    Q = -Wk[:, :, 0] / L                                     # coeff on E
    R = -Wk[:, :, 2] / L                                     # coeff on F

    Win = W_in[:, :, 0].astype(np.float64)                   # [c, c']

    def m_in(Xc):   # -> [c', k*64+c]
        return np.einsum("cp,pk->pkc", Win, Xc).reshape(C_IN, K * C_IN)

    def m_out(Xc, W):  # -> [c', o]
        return np.einsum("ock,ck->co", W.astype(np.float64), Xc)

    def mm(Xc):
        return np.concatenate([m_in(Xc), m_out(Xc, W_out)], axis=1)  # [64,320]

    m3 = np.stack([mm(P), mm(Q), mm(R)], axis=1)             # [64, 3, 320]
    mb3 = np.stack(
        [m_out(P, W_bias), m_out(Q, W_bias), m_out(R, W_bias)], axis=1
    )                                                        # [64, 3, 128]
    # mb3 feeds only the bias synthesis: bf16 keeps the PE LDWEIGHTS
    # single-pass (fp32 stationaries cost ~6x on load).  m3s/mb3s are
    # the S-coefficient blocks duplicated to 128 contract rows: stats
    # arrive as per-half partial sums on partitions 0-63 / 64-127 (the
    # fold-packed stats layout) and the duplication makes the synth
    # matmul contract S_lo + S_hi exactly.
    m3s = np.concatenate([m3[:, 0, :], m3[:, 0, :]], axis=0)
    mb3s = np.concatenate([mb3[:, 0, :], mb3[:, 0, :]], axis=0)
    return (m3.astype(np.float32), mb3.astype(ml_dtypes.bfloat16),
            m3s.astype(np.float32), mb3s.astype(ml_dtypes.bfloat16))


_CACHE = {}


def _emit_stage(nc, xp, xf, small, x_d, b, trig=None):
    """Issue x load, shifted copy, fold-packed stats load, and stats
    windows for sample b.

    The stats source is a SECOND copy of x folded in half across all
    128 partitions (first half of xpad on partitions 0-63, second half
    on 64-127): each DVE reduce window then covers both halves at once,
    halving the per-sample stats cost (DVE's stats backlog was the
    engine bound that gated every sample boundary).  S = S_lo + S_hi is
    contracted exactly by the synth matmul via duplicated S-coefficient
    rows (m3s/mb3s).  Costs one extra 1.05MB load per sample; the DMA
    engines have headroom.
    """
    H = 4097                                 # fold boundary
    xh = xp.tile([2 * C_IN, L + 2], BF16, tag="xh")
    xfold = xf.tile([2 * C_IN, H], BF16, tag="xfold")
    Sp = small.tile([2 * C_IN, 4], F32, tag="Sp")
    xe = small.tile([C_IN, 1], BF16, tag="xe")
    trig = trig if trig is not None else nc.sync
    # Trigger budget matters: each dma_start costs ~0.6-1us of its
    # engine's sequencer.  Conv-critical xh loads keep the Sync queue
    # to themselves; the fold/E stats loads ride the Scalar HWDGE at
    # the head (ACT idles there) and the GpSimd SWDGE later (Pool
    # idles mid-kernel).
    stq = nc.scalar if b == 0 else nc.gpsimd
    stq.dma_start(xe[:], x_d[b][:, L:L + 1])
    bnds = (0, 1024, 2048, 3072, H)
    if b == 0:
        # sample 0 sits on the critical ramp: the fold load goes in two
        # column-chunks so the stats reduces pipeline behind the DMA
        for lo, hi in ((0, 2048), (2048, H)):
            stq.dma_start(xfold[0:C_IN, lo:hi], x_d[b][:, lo:hi])
            stq.dma_start(xfold[C_IN:, lo:hi], x_d[b][:, H + lo:H + hi])
        lb = (0, 2048, H, 6144, L + 2)
        for c in range(4):
            trig.dma_start(xh[0:C_IN, lb[c]:lb[c + 1]],
                           x_d[b][:, lb[c]:lb[c + 1]])
        for c in range(4):
            d0, d1 = max(lb[c] - 1, 0), lb[c + 1] - 1
            trig.dma_start(xh[C_IN:, d0:d1], xh[0:C_IN, d0 + 1:d1 + 1])
    else:
        trig.dma_start(xh[0:C_IN, :], x_d[b])
        # dest col j <- src col j+1 (the shifted copy for taps 1/2)
        trig.dma_start(xh[C_IN:, 0:L + 1], xh[0:C_IN, 1:L + 2])
        stq.dma_start(xfold[0:C_IN, :], x_d[b][:, 0:H])
        stq.dma_start(xfold[C_IN:, :], x_d[b][:, H:L + 2])
    # both zero-pad columns are included: they add nothing to S.
    for c in range(4):
        nc.vector.reduce_sum(out=Sp[:, c:c + 1],
                             in_=xfold[:, bnds[c]:bnds[c + 1]],
                             axis=mybir.AxisListType.X)
    return xh, Sp, xfold, xe


def _emit_synth_steps(nc, small, pss, m3, mb3, m3s, mb3s, xh, Sp,
                      xfold, xe):
    """Stats -> (w01, w2, biasv) for one sample, as four deferred steps.

    The steps are interleaved into the PREVIOUS sample's conv matmul
    stream so the PE <-> DVE ping-pong never drains the tensor engine
    (which would also drop its p-state).  Synth PSUM packs into two
    banks (disjoint address ranges, so interleaved accumulation groups
    are safe: skip_group_check).
    """
    stat = small.tile([2 * C_IN, 3], F32R, tag="stat")
    statb = small.tile([2 * C_IN, 3], BF16, tag="statb")
    syn_pb = pss.tile([C_OUT, 512], F32, tag="syn_pb")
    syn_w = pss.tile([2 * C_IN, 256], F32, tag="syn_w")
    psp, psb = syn_pb[0:1, 0:320], syn_pb[:, 320:321]
    ps01, ps2 = syn_w[:, 0:128], syn_w[C_IN:, 128:256]
    params = small.tile([1, 320], BF16, tag="params")
    biasv = small.tile([C_OUT, 1], F32, tag="biasv")
    w01 = small.tile([2 * C_IN, C_OUT], BF16, tag="w01")
    w2 = small.tile([2 * C_IN, C_OUT], BF16, tag="w2")

    def step0():   # stats gather (DVE) + stat matmuls (PE)
        # stat col 0 holds (S_lo; S_hi) on 128 partitions; E/F live on
        # partitions 0-63 of cols 1-2.  fp32r is 32-bit in SBUF: the
        # low-precision guard is a false alarm.
        with nc.allow_low_precision(reason="fp32r out is fp32 bits"):
            nc.vector.reduce_sum(out=stat[:, 0:1], in_=Sp[:],
                                 axis=mybir.AxisListType.X)
        nc.vector.tensor_copy(stat[0:C_IN, 1:2], xe[:])                # E
        nc.vector.tensor_copy(stat[0:C_IN, 2:3], xfold[0:C_IN, 1:2])   # F
        nc.vector.tensor_copy(statb[:], stat[:].bitcast(F32))
        # S term: contract over all 128 partitions against the
        # duplicated S-coefficient rows (= S_lo + S_hi exactly)
        nc.tensor.matmul(psp, stat[:, 0:1], m3s[:], start=True,
                         stop=False, skip_group_check=True)
        nc.tensor.matmul(psb, mb3s[:], statb[:, 0:1], start=True,
                         stop=False, skip_group_check=True)
        # E/F terms: 64-contract as before.  bias chain in bf16:
        # single-pass LDWEIGHTS + matmul (the bias is one of ~190
        # additive terms in y, so bf16 rounding here is invisible)
        for j in (1, 2):
            sj = stat[0:C_IN, j:j + 1]
            nc.tensor.matmul(psp, sj, m3[:, j, :], start=False,
                             stop=(j == 2), skip_group_check=True)
            nc.tensor.matmul(psb, mb3[:, j, :], statb[0:C_IN, j:j + 1],
                             start=False, stop=(j == 2),
                             skip_group_check=True)

    def step1():
        nc.vector.tensor_copy(params[:], psp)
        nc.vector.tensor_copy(biasv[:], psb)

    def step2():
        # rank-1 stationaries: [W0; W1] on partitions 0-127, W2 on
        # 64-127.  The outers run in bf16 (w01/w2 are consumed as bf16
        # anyway): single-pass LDWEIGHTS + 1 cyc/row matmuls.
        pr = params[0:1]
        w_out_row = pr[:, 192:320]
        nc.tensor.matmul(ps01, pr[:, 0:128], w_out_row, start=True,
                         stop=True, skip_group_check=True)
        nc.tensor.matmul(ps2, pr[:, 128:192], w_out_row, start=True,
                         stop=True, skip_group_check=True)

    def step3():
        nc.vector.tensor_copy(w01[:], ps01)
        nc.vector.tensor_copy(w2[C_IN:, :], ps2)

    return (w01, w2, biasv), [step0, step1, step2, step3]


def _emit_conv(nc, yp, psy, y_d, b, xh, w01, w2, biasv, steps=()):
    """Main conv for one sample: 16 tiles x (2 matmuls, evict); 2048-col
    store chunks.  `steps` are the next sample's synth stages, dropped
    into the instruction stream mid-conv."""
    SCW = 4 * NT                             # store-chunk columns
    for g in range(NTILES // 4):
        ysb = yp.tile([C_OUT, SCW], BF16, tag="ysb")
        yo = 0
        pys = []
        # k-outer within the group: consecutive matmuls hit DIFFERENT
        # PSUM banks, so the mm2-accumulates-onto-mm1 dependency sits 4
        # matmuls back and the PE pipeline never drains (same-bank
        # back-to-back accumulation costs ~100 extra cycles per matmul).
        for i in range(4):
            py = psy.tile([C_OUT, NT], F32, tag="py")
            pys.append(py)
            m = NT * (4 * g + i)
            nc.tensor.matmul(py[:], w01[:], xh[:, m:m + NT],
                             start=True, stop=False)
        for i in range(4):
            m = NT * (4 * g + i)
            nc.tensor.matmul(pys[i][:], w2[C_IN:, :],
                             xh[C_IN:, m + 1:m + NT + 1],
                             start=False, stop=True)
        # evictions live on ACT alone: DVE holds the 8.8us/sample stats
        # reduces, and any eviction queued behind them stalls the PE on
        # PSUM banks.
        for i in range(4):
            dst = ysb[:, yo + i * NT:yo + (i + 1) * NT]
            if b == BS - 1 and i % 2 == 1:
                # DVE is idle during the last sample (no stats left):
                # take alternate evictions off ACT's critical queue
                nc.vector.tensor_scalar(dst, pys[i][:], biasv[:], None,
                                        mybir.AluOpType.add)
            else:
                nc.scalar.activation(dst, pys[i][:],
                                     mybir.ActivationFunctionType.Identity,
                                     bias=biasv[:], scale=1.0)
            # the last sample's stores are the kernel tail: drain each
            # half-chunk as soon as its evictions land
            if b == BS - 1 and i % 2 == 1:
                h0 = 4 * g * NT + (i - 1) * NT
                nc.gpsimd.dma_start(y_d[b][:, h0:h0 + 2 * NT],
                                    ysb[:, (i - 1) * NT:(i + 1) * NT])
        if b != BS - 1:
            nc.gpsimd.dma_start(y_d[b][:, 4 * g * NT:4 * g * NT + SCW],
                                ysb[:])
        if g < len(steps):
            steps[g]()


def _build_module():
    if "nc" in _CACHE:
        return _CACHE["nc"]
    nc = bacc.Bacc("TRN2", target_bir_lowering=False, debug=False)

    # host supplies x pre-padded with one zero column on each side, bf16
    x_d = nc.dram_tensor("x", [BS, C_IN, L + 2], BF16,
                         kind="ExternalInput").ap()
    m3_d = nc.dram_tensor("m3", [C_IN, 3, 320], F32R,
                          kind="ExternalInput").ap()
    mb3_d = nc.dram_tensor("mb3", [C_IN, 3, C_OUT], BF16,
                           kind="ExternalInput").ap()
    m3s_d = nc.dram_tensor("m3s", [2 * C_IN, 320], F32R,
                           kind="ExternalInput").ap()
    mb3s_d = nc.dram_tensor("mb3s", [2 * C_IN, C_OUT], BF16,
                            kind="ExternalInput").ap()
    y_d = nc.dram_tensor("y", [BS, C_OUT, L], BF16,
                         kind="ExternalOutput").ap()

    with tile.TileContext(nc) as tc:
        with (
            tc.tile_pool(name="consts", bufs=1) as consts,
            tc.tile_pool(name="xp", bufs=4) as xp,
            tc.tile_pool(name="xf", bufs=4) as xf,
            tc.tile_pool(name="yp", bufs=6) as yp,
            tc.tile_pool(name="small", bufs=2) as small,
            tc.tile_pool(name="ps_y", bufs=6, space="PSUM") as psy,
            tc.tile_pool(name="ps_s", bufs=1, space="PSUM") as pss,
        ):
            m3 = consts.tile([C_IN, 3, 320], F32R)
            mb3 = consts.tile([C_IN, 3, C_OUT], BF16)
            m3s = consts.tile([2 * C_IN, 320], F32R)
            mb3s = consts.tile([2 * C_IN, C_OUT], BF16)

            # software pipeline: stage(b) issues loads/copies/stats, synth(b)
            # runs the small fp32r matmul chain, conv(b) the 16-tile conv.
            # stage(b+2) is issued before conv(b) so its DMAs sit ahead of
            # conv(b)'s stores in the queues; stats(b+1)/(b+2) sit ahead of
            # conv(b)'s DVE evictions.  Sample 0's x load is issued before
            # the consts so it is the first transfer in the queues; consts
            # trigger from the Scalar engine to spread descriptor-gen.
            stages = {}
            stages[0] = _emit_stage(nc, xp, xf, small, x_d, 0)
            nc.scalar.dma_start(m3[:], m3_d)
            nc.scalar.dma_start(mb3[:], mb3_d)
            nc.scalar.dma_start(m3s[:], m3s_d)
            nc.scalar.dma_start(mb3s[:], mb3s_d)
            tiles0, steps0 = _emit_synth_steps(nc, small, pss, m3, mb3,
                                               m3s, mb3s, *stages[0])
            for s in steps0:           # sample 0: run synth immediately
                s()
            stages[1] = _emit_stage(nc, xp, xf, small, x_d, 1)
            synth = {0: tiles0}
            for b in range(BS):
                if b + 2 < BS:
                    stages[b + 2] = _emit_stage(nc, xp, xf, small, x_d,
                                                b + 2)
                if b + 1 < BS:
                    synth[b + 1], nxt_steps = _emit_synth_steps(
                        nc, small, pss, m3, mb3, m3s, mb3s,
                        *stages[b + 1])
                else:
                    nxt_steps = ()
                _emit_conv(nc, yp, psy, y_d, b, stages[b][0], *synth[b],
                           steps=nxt_steps)

    nc.compile()
    _CACHE["nc"] = nc
    return nc


def kernel(x, W_kernel, W_in, W_out, W_bias):
    x = np.asarray(x, dtype=np.float32)
    # one zero column each side: the device reads x[l-1], x[l], x[l+1]
    x = np.pad(x, [(0, 0), (0, 0), (1, 1)]).astype(ml_dtypes.bfloat16)
    m3, mb3, m3s, mb3s = _host_precompute(
        np.asarray(W_kernel, np.float32), np.asarray(W_in, np.float32),
        np.asarray(W_out, np.float32), np.asarray(W_bias, np.float32))

    nc = _build_module()
    in_maps = [
        {"x": x[c * BS:(c + 1) * BS], "m3": m3, "mb3": mb3,
         "m3s": m3s, "mb3s": mb3s}
        for c in range(N_CORES)
    ]
    res = run_bass_kernel_spmd(nc, in_maps, core_ids=list(range(N_CORES)))
    global LAST_RESULT
    LAST_RESULT = res
    y = np.concatenate([np.asarray(r["y"]) for r in res.results],
                       axis=0).astype(np.float32)
    return y


LAST_RESULT = None



# revision 39
# speedup vs baseline: 1.4334x; 1.4334x over previous
"""Trainium2 Bass kernel for nn_ConvPlus1d (dense_cnn).

Algorithm (mathematically identical to the reference, derived analytically):

  The reference synthesizes per-sample conv weights:
      kern[b]   = mean_L(depthwise_conv(x))        -> [B, C_IN, K]
      w_in[b]   = W_in @ kern[b]                   -> [B, C_IN, K]
      w_out[b]  = <W_out, kern[b]>                 -> [B, C_OUT]
      bias[b]   = <W_bias, kern[b]>                -> [B, C_OUT]
      weight[b, o, c, k] = w_in[b, c, k] * w_out[b, o]     (rank-1!)
      y[b] = conv1d(x[b], weight[b], pad=1) + bias[b]

  Exact simplifications:
  1) mean over L of a pad-1 depthwise conv only needs per-channel sums and
     the first/last elements:  sum_l xpad[c, l+t] = {S-E, S, S-F}[t]
     so kern / w_in / w_out / bias are LINEAR in (S, E, F), with
     coefficient matrices precomputed on the host from maker params.
  2) The per-sample conv weight is rank-1 across (o) x (c,k).

  Device program per sample (data-parallel over batch, 4 samples/core):
      x (bf16) lands in SBUF partitions 0-63; a shifted copy (one column
      left) is DMA'd into partitions 64-127.  The 3-tap conv then needs
      only TWO matmuls per 512-col tile: a 128-contract matmul computes
      taps 0+1 together (stationary [W0; W1]), a 64-contract matmul adds
      tap 2, k-outer over 4-tile groups so consecutive matmuls hit
      different PSUM banks.  Stats -> params synthesis runs in fp32r,
      interleaved into the previous sample's conv stream.  PSUM -> SBUF
      eviction adds the bias and narrows to bf16 (ACT only; DVE owns the
      stats reduces); the output is stored bf16 and widened to f32 on
      the host (~1e-3 extra relative error, half the store traffic);
      stores stream out in 2048-col chunks triggered from GpSimd.

Sharding: batch 32 -> 8 cores x 4 samples, maker params replicated.
"""

import sys

import numpy as np

sys.path.insert(0, "/opt/trn_rl_repo")

import concourse.bacc as bacc  # noqa: E402
import concourse.tile as tile  # noqa: E402
from concourse import mybir  # noqa: E402
from concourse.bass_utils import run_bass_kernel_spmd  # noqa: E402

import ml_dtypes  # noqa: E402

B, C_IN, C_OUT, K, L = 32, 64, 128, 3, 8192
N_CORES = 8
BS = B // N_CORES          # samples per core
NT = 512                   # matmul moving-dim tile (one PSUM bank of fp32)
NTILES = L // NT
NCH = 8                    # partial-reduce windows
CHW = (L + 2) // NCH       # 2048, last chunk takes the +2 remainder

F32 = mybir.dt.float32
F32R = mybir.dt.float32r
BF16 = mybir.dt.bfloat16


def _host_precompute(W_kernel, W_in, W_out, W_bias):
    """Fold the maker parameters into linear maps on the stats (S, E, F)."""
    Wk = W_kernel.reshape(C_IN, K, K).astype(np.float64)     # [c, j, t]
    P = (Wk[:, :, 0] + Wk[:, :, 1] + Wk[:, :, 2]) / L        # coeff on S
    Q = -Wk[:, :, 0] / L                                     # coeff on E
    R = -Wk[:, :, 2] / L                                     # coeff on F

    Win = W_in[:, :, 0].astype(np.float64)                   # [c, c']

    def m_in(Xc):   # -> [c', k*64+c]
        return np.einsum("cp,pk->pkc", Win, Xc).reshape(C_IN, K * C_IN)

    def m_out(Xc, W):  # -> [c', o]
        return np.einsum("ock,ck->co", W.astype(np.float64), Xc)

    def mm(Xc):
        return np.concatenate([m_in(Xc), m_out(Xc, W_out)], axis=1)  # [64,320]

    m3 = np.stack([mm(P), mm(Q), mm(R)], axis=1)             # [64, 3, 320]
    mb3 = np.stack(
        [m_out(P, W_bias), m_out(Q, W_bias), m_out(R, W_bias)], axis=1
    )                                                        # [64, 3, 128]
    # mb3 feeds only the bias synthesis: bf16 keeps the PE LDWEIGHTS
    # single-pass (fp32 stationaries cost ~6x on load)
    return m3.astype(np.float32), mb3.astype(ml_dtypes.bfloat16)


_CACHE = {}


def _emit_stage(nc, xp, small, x_d, b, trig=None):
    """Issue x load, shifted copy, and chunked stats for sample b.

    Loads and shifted copies use two big chunks (8KB per-partition rows:
    DMA queues are descriptor-rate limited, so fewer/fatter descriptors).
    Stats use four windows aligned to the load halves so each reduce
    depends on exactly one load chunk.
    """
    H = 4097                                 # load-chunk boundary
    xh = xp.tile([2 * C_IN, L + 2], BF16, tag="xh")
    Sp = small.tile([C_IN, NCH], F32, tag="Sp")
    trig = trig if trig is not None else nc.sync
    if b == 0:
        # sample 0 sits on the critical ramp: chunk the load/copy so the
        # stats reduces pipeline behind the DMA instead of after it
        lb = (0, 2048, H, 6144, L + 2)
        for c in range(4):
            trig.dma_start(xh[0:C_IN, lb[c]:lb[c + 1]],
                           x_d[b][:, lb[c]:lb[c + 1]])
        for c in range(4):
            d0, d1 = max(lb[c] - 1, 0), lb[c + 1] - 1
            trig.dma_start(xh[C_IN:, d0:d1], xh[0:C_IN, d0 + 1:d1 + 1])
        bnds = (0, 1024, 2048, 3072, H, 5120, 6144, 7169, L + 2)
    else:
        trig.dma_start(xh[0:C_IN, :], x_d[b])
        # dest col j <- src col j+1 (the shifted copy for taps 1/2)
        trig.dma_start(xh[C_IN:, 0:L + 1], xh[0:C_IN, 1:L + 2])
        bnds = (0, 1024, 2048, 3072, H, 5169, 6241, 7313, L + 2)
    # both zero-pad columns are included: they add nothing to S.
    for c in range(8):
        nc.vector.reduce_sum(out=Sp[:, c:c + 1],
                             in_=xh[0:C_IN, bnds[c]:bnds[c + 1]],
                             axis=mybir.AxisListType.X)
    return xh, Sp


def _emit_synth_steps(nc, small, pss, m3, mb3, xh, Sp):
    """Stats -> (w01, w2, biasv) for one sample, as four deferred steps.

    The steps are interleaved into the PREVIOUS sample's conv matmul
    stream so the PE <-> DVE ping-pong never drains the tensor engine
    (which would also drop its p-state).  Synth PSUM packs into two
    banks (disjoint address ranges, so interleaved accumulation groups
    are safe: skip_group_check).
    """
    stat = small.tile([C_IN, 3], F32R, tag="stat")
    statb = small.tile([C_IN, 3], BF16, tag="statb")
    syn_pb = pss.tile([C_OUT, 512], F32, tag="syn_pb")
    syn_w = pss.tile([2 * C_IN, 256], F32, tag="syn_w")
    psp, psb = syn_pb[0:1, 0:320], syn_pb[:, 320:321]
    ps01, ps2 = syn_w[:, 0:128], syn_w[C_IN:, 128:256]
    params = small.tile([1, 320], BF16, tag="params")
    biasv = small.tile([C_OUT, 1], F32, tag="biasv")
    w01 = small.tile([2 * C_IN, C_OUT], BF16, tag="w01")
    w2 = small.tile([2 * C_IN, C_OUT], BF16, tag="w2")

    def step0():   # stats gather (DVE) + stat matmuls (PE)
        # fp32r is 32-bit in SBUF: the low-precision guard is a false alarm
        with nc.allow_low_precision(reason="fp32r out is fp32 bits"):
            nc.vector.reduce_sum(out=stat[:, 0:1], in_=Sp[:],
                                 axis=mybir.AxisListType.X)
        nc.vector.tensor_copy(stat[:, 1:2], xh[0:C_IN, L:L + 1])   # E
        nc.vector.tensor_copy(stat[:, 2:3], xh[0:C_IN, 1:2])       # F
        nc.vector.tensor_copy(statb[:], stat[:].bitcast(F32))
        for j in range(3):
            sj = stat[:, j:j + 1]
            nc.tensor.matmul(psp, sj, m3[:, j, :], start=(j == 0),
                             stop=(j == 2), skip_group_check=True)
            # bias chain in bf16: single-pass LDWEIGHTS + matmul (the
            # bias is one of ~190 additive terms in y, so bf16 rounding
            # here is invisible next to the bf16 conv itself)
            nc.tensor.matmul(psb, mb3[:, j, :], statb[:, j:j + 1],
                             start=(j == 0), stop=(j == 2),
                             skip_group_check=True)

    def step1():
        nc.vector.tensor_copy(params[:], psp)
        nc.vector.tensor_copy(biasv[:], psb)

    def step2():
        # rank-1 stationaries: [W0; W1] on partitions 0-127, W2 on
        # 64-127.  The outers run in bf16 (w01/w2 are consumed as bf16
        # anyway): single-pass LDWEIGHTS + 1 cyc/row matmuls.
        pr = params[0:1]
        w_out_row = pr[:, 192:320]
        nc.tensor.matmul(ps01, pr[:, 0:128], w_out_row, start=True,
                         stop=True, skip_group_check=True)
        nc.tensor.matmul(ps2, pr[:, 128:192], w_out_row, start=True,
                         stop=True, skip_group_check=True)

    def step3():
        nc.vector.tensor_copy(w01[:], ps01)
        nc.vector.tensor_copy(w2[C_IN:, :], ps2)

    return (w01, w2, biasv), [step0, step1, step2, step3]


def _emit_conv(nc, yp, psy, y_d, b, xh, w01, w2, biasv, steps=()):
    """Main conv for one sample: 16 tiles x (2 matmuls, evict); 2048-col
    store chunks.  `steps` are the next sample's synth stages, dropped
    into the instruction stream mid-conv."""
    SCW = 4 * NT                             # store-chunk columns
    for g in range(NTILES // 4):
        ysb = yp.tile([C_OUT, SCW], BF16, tag="ysb")
        yo = 0
        pys = []
        # k-outer within the group: consecutive matmuls hit DIFFERENT
        # PSUM banks, so the mm2-accumulates-onto-mm1 dependency sits 4
        # matmuls back and the PE pipeline never drains (same-bank
        # back-to-back accumulation costs ~100 extra cycles per matmul).
        for i in range(4):
            py = psy.tile([C_OUT, NT], F32, tag="py")
            pys.append(py)
            m = NT * (4 * g + i)
            nc.tensor.matmul(py[:], w01[:], xh[:, m:m + NT],
                             start=True, stop=False)
        for i in range(4):
            m = NT * (4 * g + i)
            nc.tensor.matmul(pys[i][:], w2[C_IN:, :],
                             xh[C_IN:, m + 1:m + NT + 1],
                             start=False, stop=True)
        # evictions live on ACT alone: DVE holds the 8.8us/sample stats
        # reduces, and any eviction queued behind them stalls the PE on
        # PSUM banks.
        for i in range(4):
            dst = ysb[:, yo + i * NT:yo + (i + 1) * NT]
            if b == BS - 1 and i % 2 == 1:
                # DVE is idle during the last sample (no stats left):
                # take alternate evictions off ACT's critical queue
                nc.vector.tensor_scalar(dst, pys[i][:], biasv[:], None,
                                        mybir.AluOpType.add)
            else:
                nc.scalar.activation(dst, pys[i][:],
                                     mybir.ActivationFunctionType.Identity,
                                     bias=biasv[:], scale=1.0)
            # the last sample's stores are the kernel tail: drain each
            # half-chunk as soon as its evictions land
            if b == BS - 1 and i % 2 == 1:
                h0 = 4 * g * NT + (i - 1) * NT
                nc.gpsimd.dma_start(y_d[b][:, h0:h0 + 2 * NT],
                                    ysb[:, (i - 1) * NT:(i + 1) * NT])
        if b != BS - 1:
            nc.gpsimd.dma_start(y_d[b][:, 4 * g * NT:4 * g * NT + SCW],
                                ysb[:])
        if g < len(steps):
            steps[g]()


def _build_module():
    if "nc" in _CACHE:
        return _CACHE["nc"]
    nc = bacc.Bacc("TRN2", target_bir_lowering=False, debug=False)

    # host supplies x pre-padded with one zero column on each side, bf16
    x_d = nc.dram_tensor("x", [BS, C_IN, L + 2], BF16,
                         kind="ExternalInput").ap()
    m3_d = nc.dram_tensor("m3", [C_IN, 3, 320], F32R,
                          kind="ExternalInput").ap()
    mb3_d = nc.dram_tensor("mb3", [C_IN, 3, C_OUT], BF16,
                           kind="ExternalInput").ap()
    y_d = nc.dram_tensor("y", [BS, C_OUT, L], BF16,
                         kind="ExternalOutput").ap()

    with tile.TileContext(nc) as tc:
        with (
            tc.tile_pool(name="consts", bufs=1) as consts,
            tc.tile_pool(name="xp", bufs=4) as xp,
            tc.tile_pool(name="yp", bufs=6) as yp,
            tc.tile_pool(name="small", bufs=2) as small,
            tc.tile_pool(name="ps_y", bufs=6, space="PSUM") as psy,
            tc.tile_pool(name="ps_s", bufs=1, space="PSUM") as pss,
        ):
            m3 = consts.tile([C_IN, 3, 320], F32R)
            mb3 = consts.tile([C_IN, 3, C_OUT], BF16)

            # software pipeline: stage(b) issues loads/copies/stats, synth(b)
            # runs the small fp32r matmul chain, conv(b) the 16-tile conv.
            # stage(b+2) is issued before conv(b) so its DMAs sit ahead of
            # conv(b)'s stores in the queues; stats(b+1)/(b+2) sit ahead of
            # conv(b)'s DVE evictions.  Sample 0's x load is issued before
            # the consts so it is the first transfer in the queues; consts
            # trigger from the Scalar engine to spread descriptor-gen.
            stages = {}
            stages[0] = _emit_stage(nc, xp, small, x_d, 0)
            nc.scalar.dma_start(m3[:], m3_d)
            nc.scalar.dma_start(mb3[:], mb3_d)
            tiles0, steps0 = _emit_synth_steps(nc, small, pss, m3, mb3,
                                               *stages[0])
            for s in steps0:           # sample 0: run synth immediately
                s()
            stages[1] = _emit_stage(nc, xp, small, x_d, 1)
            synth = {0: tiles0}
            for b in range(BS):
                if b + 2 < BS:
                    stages[b + 2] = _emit_stage(nc, xp, small, x_d, b + 2)
                if b + 1 < BS:
                    synth[b + 1], nxt_steps = _emit_synth_steps(
                        nc, small, pss, m3, mb3, *stages[b + 1])
                else:
                    nxt_steps = ()
                _emit_conv(nc, yp, psy, y_d, b, stages[b][0], *synth[b],
                           steps=nxt_steps)

    nc.compile()
    _CACHE["nc"] = nc
    return nc


def kernel(x, W_kernel, W_in, W_out, W_bias):
    x = np.asarray(x, dtype=np.float32)
    # one zero column each side: the device reads x[l-1], x[l], x[l+1]
    x = np.pad(x, [(0, 0), (0, 0), (1, 1)]).astype(ml_dtypes.bfloat16)
    m3, mb3 = _host_precompute(
        np.asarray(W_kernel, np.float32), np.asarray(W_in, np.float32),
        np.asarray(W_out, np.float32), np.asarray(W_bias, np.float32))

    nc = _build_module()
    in_maps = [
        {"x": x[c * BS:(c + 1) * BS], "m3": m3, "mb3": mb3}
        for c in range(N_CORES)
    ]
    res = run_bass_kernel_spmd(nc, in_maps, core_ids=list(range(N_CORES)))
    global LAST_RESULT
    LAST_RESULT = res
    y = np.concatenate([np.asarray(r["y"]) for r in res.results],
                       axis=0).astype(np.float32)
    return y


LAST_RESULT = None



# revision 40
# speedup vs baseline: 1.4955x; 1.0433x over previous
"""Trainium2 Bass kernel for nn_ConvPlus1d (dense_cnn).

Algorithm (mathematically identical to the reference, derived analytically):

  The reference synthesizes per-sample conv weights:
      kern[b]   = mean_L(depthwise_conv(x))        -> [B, C_IN, K]
      w_in[b]   = W_in @ kern[b]                   -> [B, C_IN, K]
      w_out[b]  = <W_out, kern[b]>                 -> [B, C_OUT]
      bias[b]   = <W_bias, kern[b]>                -> [B, C_OUT]
      weight[b, o, c, k] = w_in[b, c, k] * w_out[b, o]     (rank-1!)
      y[b] = conv1d(x[b], weight[b], pad=1) + bias[b]

  Exact simplifications:
  1) mean over L of a pad-1 depthwise conv only needs per-channel sums and
     the first/last elements:  sum_l xpad[c, l+t] = {S-E, S, S-F}[t]
     so kern / w_in / w_out / bias are LINEAR in (S, E, F), with
     coefficient matrices precomputed on the host from maker params.
  2) The per-sample conv weight is rank-1 across (o) x (c,k).

  Device program per sample (data-parallel over batch, 4 samples/core):
      x (bf16) lands in SBUF partitions 0-63; a shifted copy (one column
      left) is DMA'd into partitions 64-127.  The 3-tap conv then needs
      only TWO matmuls per 512-col tile: a 128-contract matmul computes
      taps 0+1 together (stationary [W0; W1]), a 64-contract matmul adds
      tap 2, k-outer over 4-tile groups so consecutive matmuls hit
      different PSUM banks.  Stats -> params synthesis runs in fp32r,
      interleaved into the previous sample's conv stream.  PSUM -> SBUF
      eviction adds the bias and narrows to bf16 (ACT only; DVE owns the
      stats reduces); the output is stored bf16 and widened to f32 on
      the host (~1e-3 extra relative error, half the store traffic);
      stores stream out in 2048-col chunks triggered from GpSimd.

Sharding: batch 32 -> 8 cores x 4 samples, maker params replicated.
"""

import sys

import numpy as np

sys.path.insert(0, "/opt/trn_rl_repo")

import concourse.bacc as bacc  # noqa: E402
import concourse.tile as tile  # noqa: E402
from concourse import mybir  # noqa: E402
from concourse.bass_utils import run_bass_kernel_spmd  # noqa: E402

import ml_dtypes  # noqa: E402

B, C_IN, C_OUT, K, L = 32, 64, 128, 3, 8192
N_CORES = 8
BS = B // N_CORES          # samples per core
NT = 512                   # matmul moving-dim tile (one PSUM bank of fp32)
NTILES = L // NT
NCH = 8                    # partial-reduce windows
CHW = (L + 2) // NCH       # 2048, last chunk takes the +2 remainder

F32 = mybir.dt.float32
F32R = mybir.dt.float32r
BF16 = mybir.dt.bfloat16


def _host_precompute(W_kernel, W_in, W_out, W_bias):
    """Fold the maker parameters into linear maps on the stats (S, E, F)."""
    Wk = W_kernel.reshape(C_IN, K, K).astype(np.float64)     # [c, j, t]
    P = (Wk[:, :, 0] + Wk[:, :, 1] + Wk[:, :, 2]) / L        # coeff on S
    Q = -Wk[:, :, 0] / L                                     # coeff on E
    R = -Wk[:, :, 2] / L                                     # coeff on F

    Win = W_in[:, :, 0].astype(np.float64)                   # [c, c']

    def m_in(Xc):   # -> [c', k*64+c]
        return np.einsum("cp,pk->pkc", Win, Xc).reshape(C_IN, K * C_IN)

    def m_out(Xc, W):  # -> [c', o]
        return np.einsum("ock,ck->co", W.astype(np.float64), Xc)

    def mm(Xc):
        return np.concatenate([m_in(Xc), m_out(Xc, W_out)], axis=1)  # [64,320]

    m3 = np.stack([mm(P), mm(Q), mm(R)], axis=1)             # [64, 3, 320]
    mb3 = np.stack(
        [m_out(P, W_bias), m_out(Q, W_bias), m_out(R, W_bias)], axis=1
    )                                                        # [64, 3, 128]
    # mb3 feeds only the bias synthesis: bf16 keeps the PE LDWEIGHTS
    # single-pass (fp32 stationaries cost ~6x on load)
    return m3.astype(np.float32), mb3.astype(ml_dtypes.bfloat16)


_CACHE = {}


def _emit_stage(nc, xp, small, x_d, b, trig=None):
    """Issue x load, shifted copy, and chunked stats for sample b.

    Loads and shifted copies use two big chunks (8KB per-partition rows:
    DMA queues are descriptor-rate limited, so fewer/fatter descriptors).
    Stats use four windows aligned to the load halves so each reduce
    depends on exactly one load chunk.
    """
    H = 4097                                 # load-chunk boundary
    xh = xp.tile([2 * C_IN, L + 2], BF16, tag="xh")
    Sp = small.tile([C_IN, NCH], F32, tag="Sp")
    trig = trig if trig is not None else nc.sync
    if b == 0:
        # sample 0 sits on the critical ramp: chunk the load/copy so the
        # stats reduces pipeline behind the DMA instead of after it
        lb = (0, 2048, H, 6144, L + 2)
        for c in range(4):
            trig.dma_start(xh[0:C_IN, lb[c]:lb[c + 1]],
                           x_d[b][:, lb[c]:lb[c + 1]])
        for c in range(4):
            d0, d1 = max(lb[c] - 1, 0), lb[c + 1] - 1
            trig.dma_start(xh[C_IN:, d0:d1], xh[0:C_IN, d0 + 1:d1 + 1])
        bnds = (0, 1024, 2048, 3072, H, 5120, 6144, 7169, L + 2)
    else:
        trig.dma_start(xh[0:C_IN, :], x_d[b])
        # dest col j <- src col j+1 (the shifted copy for taps 1/2)
        trig.dma_start(xh[C_IN:, 0:L + 1], xh[0:C_IN, 1:L + 2])
        bnds = (0, 1024, 2048, 3072, H, 5169, 6241, 7313, L + 2)
    # both zero-pad columns are included: they add nothing to S.
    for c in range(8):
        nc.vector.reduce_sum(out=Sp[:, c:c + 1],
                             in_=xh[0:C_IN, bnds[c]:bnds[c + 1]],
                             axis=mybir.AxisListType.X)
    return xh, Sp


def _emit_synth_steps(nc, small, pss, m3, mb3, xh, Sp):
    """Stats -> (w01, w2, biasv) for one sample, as four deferred steps.

    The steps are interleaved into the PREVIOUS sample's conv matmul
    stream so the PE <-> DVE ping-pong never drains the tensor engine
    (which would also drop its p-state).  Synth PSUM packs into two
    banks (disjoint address ranges, so interleaved accumulation groups
    are safe: skip_group_check).
    """
    stat = small.tile([C_IN, 3], F32R, tag="stat")
    statb = small.tile([C_IN, 3], BF16, tag="statb")
    # ONE PSUM bank holds the whole synth: ps01 on cols 0-127 (all
    # partitions), ps2 on cols 128-255 (partitions 64-127 only), psp on
    # cols 128-447 (partition 0 only -- disjoint partitions from ps2),
    # psb on col 448.  That frees a 7th bank for the conv rotation, so
    # adjacent 4-tile groups share one bank instead of two (fewer PSUM
    # waits -> steadier PE -> earlier p-state ramp).
    syn = pss.tile([2 * C_IN, 512], F32, tag="syn")
    ps01, ps2 = syn[:, 0:128], syn[C_IN:, 128:256]
    psp, psb = syn[0:1, 128:448], syn[:, 448:449]
    params = small.tile([1, 320], BF16, tag="params")
    biasv = small.tile([C_OUT, 1], F32, tag="biasv")
    w01 = small.tile([2 * C_IN, C_OUT], BF16, tag="w01")
    w2 = small.tile([2 * C_IN, C_OUT], BF16, tag="w2")

    def step0():   # stats gather (DVE) + stat matmuls (PE)
        # fp32r is 32-bit in SBUF: the low-precision guard is a false alarm
        with nc.allow_low_precision(reason="fp32r out is fp32 bits"):
            nc.vector.reduce_sum(out=stat[:, 0:1], in_=Sp[:],
                                 axis=mybir.AxisListType.X)
        nc.vector.tensor_copy(stat[:, 1:2], xh[0:C_IN, L:L + 1])   # E
        nc.vector.tensor_copy(stat[:, 2:3], xh[0:C_IN, 1:2])       # F
        nc.vector.tensor_copy(statb[:], stat[:].bitcast(F32))
        for j in range(3):
            sj = stat[:, j:j + 1]
            nc.tensor.matmul(psp, sj, m3[:, j, :], start=(j == 0),
                             stop=(j == 2), skip_group_check=True)
            # bias chain in bf16: single-pass LDWEIGHTS + matmul (the
            # bias is one of ~190 additive terms in y, so bf16 rounding
            # here is invisible next to the bf16 conv itself)
            nc.tensor.matmul(psb, mb3[:, j, :], statb[:, j:j + 1],
                             start=(j == 0), stop=(j == 2),
                             skip_group_check=True)

    def step1():
        nc.vector.tensor_copy(params[:], psp)
        nc.vector.tensor_copy(biasv[:], psb)

    def step2():
        # rank-1 stationaries: [W0; W1] on partitions 0-127, W2 on
        # 64-127.  The outers run in bf16 (w01/w2 are consumed as bf16
        # anyway): single-pass LDWEIGHTS + 1 cyc/row matmuls.
        pr = params[0:1]
        w_out_row = pr[:, 192:320]
        nc.tensor.matmul(ps01, pr[:, 0:128], w_out_row, start=True,
                         stop=True, skip_group_check=True)
        nc.tensor.matmul(ps2, pr[:, 128:192], w_out_row, start=True,
                         stop=True, skip_group_check=True)

    def step3():
        nc.vector.tensor_copy(w01[:], ps01)
        nc.vector.tensor_copy(w2[C_IN:, :], ps2)

    return (w01, w2, biasv), [step0, step1, step2, step3]


def _emit_conv(nc, yp, psy, y_d, b, xh, w01, w2, biasv, steps=()):
    """Main conv for one sample: 16 tiles x (2 matmuls, evict); 2048-col
    store chunks.  `steps` are the next sample's synth stages, dropped
    into the instruction stream mid-conv."""
    SCW = 4 * NT                             # store-chunk columns
    for g in range(NTILES // 4):
        ysb = yp.tile([C_OUT, SCW], BF16, tag="ysb")
        yo = 0
        pys = []
        # k-outer within the group: consecutive matmuls hit DIFFERENT
        # PSUM banks, so the mm2-accumulates-onto-mm1 dependency sits 4
        # matmuls back and the PE pipeline never drains (same-bank
        # back-to-back accumulation costs ~100 extra cycles per matmul).
        for i in range(4):
            py = psy.tile([C_OUT, NT], F32, tag="py")
            pys.append(py)
            m = NT * (4 * g + i)
            nc.tensor.matmul(py[:], w01[:], xh[:, m:m + NT],
                             start=True, stop=False)
        for i in range(4):
            m = NT * (4 * g + i)
            nc.tensor.matmul(pys[i][:], w2[C_IN:, :],
                             xh[C_IN:, m + 1:m + NT + 1],
                             start=False, stop=True)
        # evictions live on ACT alone: DVE holds the 8.8us/sample stats
        # reduces, and any eviction queued behind them stalls the PE on
        # PSUM banks.
        for i in range(4):
            dst = ysb[:, yo + i * NT:yo + (i + 1) * NT]
            if b == BS - 1 and i % 2 == 1:
                # DVE is idle during the last sample (no stats left):
                # take alternate evictions off ACT's critical queue
                nc.vector.tensor_scalar(dst, pys[i][:], biasv[:], None,
                                        mybir.AluOpType.add)
            else:
                nc.scalar.activation(dst, pys[i][:],
                                     mybir.ActivationFunctionType.Identity,
                                     bias=biasv[:], scale=1.0)
            # the last sample's stores are the kernel tail: drain each
            # half-chunk as soon as its evictions land
            if b == BS - 1 and i % 2 == 1:
                h0 = 4 * g * NT + (i - 1) * NT
                nc.gpsimd.dma_start(y_d[b][:, h0:h0 + 2 * NT],
                                    ysb[:, (i - 1) * NT:(i + 1) * NT])
        if b != BS - 1:
            nc.gpsimd.dma_start(y_d[b][:, 4 * g * NT:4 * g * NT + SCW],
                                ysb[:])
        if g < len(steps):
            steps[g]()


def _build_module():
    if "nc" in _CACHE:
        return _CACHE["nc"]
    nc = bacc.Bacc("TRN2", target_bir_lowering=False, debug=False)

    # host supplies x pre-padded with one zero column on each side, bf16
    x_d = nc.dram_tensor("x", [BS, C_IN, L + 2], BF16,
                         kind="ExternalInput").ap()
    m3_d = nc.dram_tensor("m3", [C_IN, 3, 320], F32R,
                          kind="ExternalInput").ap()
    mb3_d = nc.dram_tensor("mb3", [C_IN, 3, C_OUT], BF16,
                           kind="ExternalInput").ap()
    y_d = nc.dram_tensor("y", [BS, C_OUT, L], BF16,
                         kind="ExternalOutput").ap()

    with tile.TileContext(nc) as tc:
        with (
            tc.tile_pool(name="consts", bufs=1) as consts,
            tc.tile_pool(name="xp", bufs=4) as xp,
            tc.tile_pool(name="yp", bufs=6) as yp,
            tc.tile_pool(name="small", bufs=2) as small,
            tc.tile_pool(name="ps_y", bufs=7, space="PSUM") as psy,
            tc.tile_pool(name="ps_s", bufs=1, space="PSUM") as pss,
        ):
            m3 = consts.tile([C_IN, 3, 320], F32R)
            mb3 = consts.tile([C_IN, 3, C_OUT], BF16)

            # software pipeline: stage(b) issues loads/copies/stats, synth(b)
            # runs the small fp32r matmul chain, conv(b) the 16-tile conv.
            # stage(b+2) is issued before conv(b) so its DMAs sit ahead of
            # conv(b)'s stores in the queues; stats(b+1)/(b+2) sit ahead of
            # conv(b)'s DVE evictions.  Sample 0's x load is issued before
            # the consts so it is the first transfer in the queues; consts
            # trigger from the Scalar engine to spread descriptor-gen.
            stages = {}
            stages[0] = _emit_stage(nc, xp, small, x_d, 0)
            nc.scalar.dma_start(m3[:], m3_d)
            nc.scalar.dma_start(mb3[:], mb3_d)
            tiles0, steps0 = _emit_synth_steps(nc, small, pss, m3, mb3,
                                               *stages[0])
            for s in steps0:           # sample 0: run synth immediately
                s()
            stages[1] = _emit_stage(nc, xp, small, x_d, 1)
            synth = {0: tiles0}
            for b in range(BS):
                if b + 2 < BS:
                    stages[b + 2] = _emit_stage(nc, xp, small, x_d, b + 2)
                if b + 1 < BS:
                    synth[b + 1], nxt_steps = _emit_synth_steps(
                        nc, small, pss, m3, mb3, *stages[b + 1])
                else:
                    nxt_steps = ()
                _emit_conv(nc, yp, psy, y_d, b, stages[b][0], *synth[b],
                           steps=nxt_steps)

    nc.compile()
    _CACHE["nc"] = nc
    return nc


def kernel(x, W_kernel, W_in, W_out, W_bias):
    x = np.asarray(x, dtype=np.float32)
    # one zero column each side: the device reads x[l-1], x[l], x[l+1]
    x = np.pad(x, [(0, 0), (0, 0), (1, 1)]).astype(ml_dtypes.bfloat16)
    m3, mb3 = _host_precompute(
        np.asarray(W_kernel, np.float32), np.asarray(W_in, np.float32),
        np.asarray(W_out, np.float32), np.asarray(W_bias, np.float32))

    nc = _build_module()
    in_maps = [
        {"x": x[c * BS:(c + 1) * BS], "m3": m3, "mb3": mb3}
        for c in range(N_CORES)
    ]
    res = run_bass_kernel_spmd(nc, in_maps, core_ids=list(range(N_CORES)))
    global LAST_RESULT
    LAST_RESULT = res
    y = np.concatenate([np.asarray(r["y"]) for r in res.results],
                       axis=0).astype(np.float32)
    return y


LAST_RESULT = None

